# revision 7
# baseline (speedup 1.0000x reference)
"""Trainium2 Bass kernel for nn_ClassificationHead.

Math (per batch b, query q):
  qe        = query_table[label_ids[b,q]]                      (E,)
  gamma,beta= split(qe @ W_film + b_film)                      (C,), (C,)
  film[n,c] = f[n,c]*gamma[c] + beta[c]      f = feature[b] as (HW, C)
  hidden    = gelu(film @ W_att_h + b_att_h)                   (HW, H)
  attn      = sigmoid(hidden @ W_att_f + b_att_f)              (HW, 1)
  pooled    = sum_n attn*film / (sum attn + 1e-8)              (C,)
  logits    = gelu(pooled @ W_mlp1 + b_mlp1) @ W_mlp2 + b_mlp2 (NCLS,)

Restructure used on device (film is never materialized):
  hiddenT[h,n] = gelu( sum_c (gamma[c]*W_att_h[c,h]) * fT[c,n] + bh[h] )
      with bh[h] = b_att_h[h] + sum_c beta[c]*W_att_h[c,h]
  attn = 0.5 + 0.5*u,  uT[n] = tanh( (hiddenT[:,n] . 0.5*W_att_f) + 0.5*b_att_f )
      computed directly in transposed layout: per n-chunk i a matmul with
      lhsT = hiddenT[:, i*128:(i+1)*128] writes psum column q of uT_ps[i]
  pooled[c] = gamma[c]*(colsum_f[c] + t[c]) * a  +  beta[c]
      t[c] = sum_n f[n,c]*u[n],  a = 1/(HW + sum_n u[n] + 2e-8),
      colsum_f[c] = sum_n fT[c,n]
  (the beta ratio (HW+S)/(HW+S+2e-8) rounds to exactly 1.0 in fp32, as it
   does in the fp32 reference, so it is dropped)

Sharding: data-parallel over B across the 8 cores (B == 8).
Both gelu (hidden + MLP) and tanh come from the single ACT table set
`gelu_and_others`, so there is exactly one activation-table load.
All matmul inputs are bf16 (fp32 LDWEIGHTS runs two passes and is ~6x
slower); accumulation is always fp32 in PSUM.  The per-q gamma*W scaling
is split 2 chunks on DVE + 2 on GpSimd so neither engine gates the PE.
"""

import os
import numpy as np

B = 8
Q = 64
C = 512
HW = 256
E = 512
HID = 128       # HID_ATT
HID2 = 512      # HID_MLP
NCLS = 14
NLAB = 64
NCORES = 8

_CACHE = {}
LAST_EXEC_NS = None
LAST_TRACE = None


def _build_nc():
    import concourse.bass as bass
    import concourse.tile as tile
    from concourse import bacc, mybir

    f32 = mybir.dt.float32
    bf16 = mybir.dt.bfloat16
    A = mybir.ActivationFunctionType
    AX = mybir.AxisListType

    nc = bacc.Bacc("TRN2", target_bir_lowering=False, debug=False,
                   num_devices=NCORES)

    # ---- DRAM tensors (per-core inputs) ----
    feat_d = nc.dram_tensor("feat", [C, HW], f32, kind="ExternalInput")
    ohb_d = nc.dram_tensor("ohb", [NLAB, Q], bf16, kind="ExternalInput")
    qtb_d = nc.dram_tensor("qtb", [NLAB, E], bf16, kind="ExternalInput")
    wfilmb_d = nc.dram_tensor("wfilmb", [E, 2 * C], bf16, kind="ExternalInput")
    bfilmb_d = nc.dram_tensor("bfilmb", [1, 2 * C], bf16, kind="ExternalInput")
    wahb_d = nc.dram_tensor("wahb", [C, HID], bf16, kind="ExternalInput")
    bahb_d = nc.dram_tensor("bahb", [1, HID], bf16, kind="ExternalInput")
    wafhb_d = nc.dram_tensor("wafhb", [HID, 1], bf16, kind="ExternalInput")
    baf_d = nc.dram_tensor("baf", [1, 1], f32, kind="ExternalInput")
    wm1b_d = nc.dram_tensor("wm1b", [C, HID2], bf16, kind="ExternalInput")
    bm1_d = nc.dram_tensor("bm1", [HID2], f32, kind="ExternalInput")
    wm2b_d = nc.dram_tensor("wm2b", [C, NCLS], bf16, kind="ExternalInput")
    bm2b_d = nc.dram_tensor("bm2b", [1, NCLS], bf16, kind="ExternalInput")
    out_d = nc.dram_tensor("out", [Q, NCLS], f32, kind="ExternalOutput")

    KC = C // 128   # 4 c-chunks
    KN = HW // 128  # 2 n-chunks

    with tile.TileContext(nc) as tc:
        with (
            tc.tile_pool(name="const", bufs=1) as const,
            tc.tile_pool(name="work", bufs=3) as work,
            tc.tile_pool(name="psmisc", bufs=3, space="PSUM") as psm,
            tc.tile_pool(name="pshid", bufs=2, space="PSUM") as psh,
            tc.tile_pool(name="psu", bufs=1, space="PSUM") as psu,
        ):
            # ---------------- DMAs in ----------------
            qtb_sb = const.tile([NLAB, E], bf16)
            nc.sync.dma_start(qtb_sb[:], qtb_d[:])
            ohb_sb = const.tile([NLAB, Q], bf16)
            nc.sync.dma_start(ohb_sb[:], ohb_d[:])
            bfilmb_sb = const.tile([1, 2 * C], bf16)
            nc.sync.dma_start(bfilmb_sb[:], bfilmb_d[:])
            wahb_sb = const.tile([128, KC, HID], bf16)
            nc.sync.dma_start(
                wahb_sb[:], wahb_d.ap().rearrange("(k p) h -> p k h", p=128))
            bahb_sb = const.tile([1, HID], bf16)
            nc.sync.dma_start(bahb_sb[:], bahb_d[:])
            wafhb_sb = const.tile([HID, 1], bf16)
            nc.sync.dma_start(wafhb_sb[:], wafhb_d[:])
            baf_sb = const.tile([1, 1], f32)
            nc.sync.dma_start(baf_sb[:], baf_d[:])
            wfb_sb = []
            for k in range(KC):
                t = const.tile([128, 2 * C], bf16, tag=f"wf{k}")
                nc.sync.dma_start(t[:], wfilmb_d[k * 128:(k + 1) * 128, :])
                wfb_sb.append(t)
            feat_sb = const.tile([128, KC, HW], f32)
            nc.sync.dma_start(
                feat_sb[:], feat_d.ap().rearrange("(k p) n -> p k n", p=128))
            wm1b_sb = const.tile([128, KC, HID2], bf16)
            nc.sync.dma_start(
                wm1b_sb[:], wm1b_d.ap().rearrange("(k p) m -> p k m", p=128))
            bm1_sb = const.tile([128, KC], f32)
            nc.sync.dma_start(
                bm1_sb[:], bm1_d.ap().rearrange("(k p) -> p k", p=128))
            wm2b_sb = const.tile([128, KC, NCLS], bf16)
            nc.sync.dma_start(
                wm2b_sb[:], wm2b_d.ap().rearrange("(k p) o -> p k o", p=128))
            bm2b_sb = const.tile([1, NCLS], bf16)
            nc.sync.dma_start(bm2b_sb[:], bm2b_d[:])

            # ---------------- constants ----------------
            ones_b = const.tile([1, 128], bf16)
            nc.vector.memset(ones_b[:], 1.0)
            half_row = const.tile([1, 128], f32)
            nc.vector.memset(half_row[:], 0.5)
            ones_col_b = const.tile([128, 1], bf16)
            nc.vector.memset(ones_col_b[:], 1.0)
            ones_row = const.tile([1, 128], f32)
            nc.vector.memset(ones_row[:], 1.0)

            # ---------------- setup compute ----------------
            # qe^T = query_table^T gathered by onehot: [e, q] in 4 chunks
            qeT = const.tile([128, KC, Q], bf16)
            for k in range(KC):
                ps = psm.tile([128, Q], f32, tag="m")
                nc.tensor.matmul(ps[:], qtb_sb[:, k * 128:(k + 1) * 128],
                                 ohb_sb[:], start=True, stop=True)
                nc.vector.tensor_copy(qeT[:, k, :], ps[:])

            # gb^T = W_film^T @ qe^T + b_film  -> gammaT, betaT  [c, q] (fp32)
            gammaT = const.tile([128, KC, Q], f32)
            betaT = const.tile([128, KC, Q], f32)
            for j in range(2 * KC):
                ps = psm.tile([128, Q], f32, tag="m")
                for k in range(KC):
                    nc.tensor.matmul(ps[:],
                                     wfb_sb[k][:, j * 128:(j + 1) * 128],
                                     qeT[:, k, :],
                                     start=(k == 0), stop=False)
                nc.tensor.matmul(ps[:], bfilmb_sb[:, j * 128:(j + 1) * 128],
                                 ones_b[:, :Q], start=False, stop=True)
                if j < KC:
                    nc.vector.tensor_copy(gammaT[:, j, :], ps[:])
                else:
                    nc.scalar.copy(betaT[:, j - KC, :], ps[:])
            betab = const.tile([128, KC, Q], bf16)
            nc.vector.tensor_copy(betab[:], betaT[:])

            # bh[h, q] = b_att_h[h] + sum_c beta[c,q] W_att_h[c,h]
            bh_sb = const.tile([HID, Q], f32)
            ps_bh = psm.tile([HID, Q], f32, tag="m")
            for k in range(KC):
                nc.tensor.matmul(ps_bh[:], wahb_sb[:, k, :], betab[:, k, :],
                                 start=(k == 0), stop=False)
            nc.tensor.matmul(ps_bh[:], bahb_sb[:], ones_b[:, :Q],
                             start=False, stop=True)
            nc.vector.tensor_copy(bh_sb[:], ps_bh[:])

            # 0.5*b_att_f broadcast down 128 partitions
            bfa_half = const.tile([128, 1], f32)
            ps_bf = psm.tile([128, 1], f32, tag="m")
            nc.tensor.matmul(ps_bf[:], half_row[:], baf_sb[:],
                             start=True, stop=True)
            nc.vector.tensor_copy(bfa_half[:], ps_bf[:])

            # bf16 copy of f^T; exact column sums of f^T (fp32)
            ftb = const.tile([128, KC, HW], bf16)
            nc.vector.tensor_copy(ftb[:], feat_sb[:])
            colsum = const.tile([128, KC], f32)
            for k in range(KC):
                nc.vector.reduce_sum(colsum[:, k:k + 1], feat_sb[:, k, :],
                                     axis=AX.X)

            # f (HW-major, bf16) via SBUF->SBUF DMA transposes
            f_sb = []
            for i in range(KN):
                t = const.tile([128, C], bf16, tag=f"fsb{i}")
                f_sb.append(t)
            for i in range(KN):
                for k in range(KC):
                    nc.sync.dma_start_transpose(
                        f_sb[i][:, k * 128:(k + 1) * 128],
                        ftb[:, k, i * 128:(i + 1) * 128])

            # ---------------- main loop over queries ----------------
            # uT_ps[i][n, q] accumulates attention-logit columns directly in
            # transposed layout (free-dim offset q is unrestricted).
            uT_ps0 = psu.tile([128, Q], f32, tag="u0")
            uT_ps1 = psu.tile([128, Q], f32, tag="u1")
            uT_ps = [uT_ps0, uT_ps1]
            for q in range(Q):
                wq = work.tile([128, KC, HID], bf16, tag="wq")
                for k in range(KC):
                    eng = nc.vector if k < 2 else nc.gpsimd
                    eng.tensor_scalar_mul(
                        wq[:, k, :], wahb_sb[:, k, :], gammaT[:, k, q:q + 1])
                hid_ps = psh.tile([HID, HW], f32, tag="h")
                for k in range(KC):
                    nc.tensor.matmul(hid_ps[:], wq[:, k, :], ftb[:, k, :],
                                     start=(k == 0), stop=(k == KC - 1))
                hidT = work.tile([HID, HW], bf16, tag="hidT")
                nc.scalar.activation(hidT[:], hid_ps[:], A.Gelu,
                                     bias=bh_sb[:, q:q + 1], scale=1.0)
                for i in range(KN):
                    nc.tensor.matmul(uT_ps[i][:, q:q + 1],
                                     hidT[:, i * 128:(i + 1) * 128],
                                     wafhb_sb[:], start=True, stop=True)

            # ---------------- pooling tail ----------------
            # u = tanh(0.5 z + 0.5 b_att_f), straight from PSUM, bf16
            uT = const.tile([128, KN, Q], bf16)
            for i in range(KN):
                nc.scalar.activation(uT[:, i, :], uT_ps[i][:], A.Tanh,
                                     bias=bfa_half[:], scale=1.0)

            # t[c, q] = sum_n f[n,c] u[n,q];  S[q] = sum_n u[n,q]
            s_ps = psm.tile([1, Q], f32, tag="m")
            for i in range(KN):
                nc.tensor.matmul(s_ps[:], ones_col_b[:], uT[:, i, :],
                                 start=(i == 0), stop=(i == KN - 1))
            den = const.tile([1, Q], f32)
            nc.vector.tensor_scalar_add(den[:], s_ps[:], float(HW) + 2e-8)
            inv = const.tile([1, 128], f32)
            nc.vector.reciprocal(inv[:, :Q], den[:])
            # broadcast a = inv down partitions
            bc_ps = psm.tile([128, Q], f32, tag="m")
            nc.tensor.matmul(bc_ps[:], ones_row[:], inv[:, :Q],
                             start=True, stop=True)
            a_bc = const.tile([128, Q], f32)
            nc.vector.tensor_copy(a_bc[:], bc_ps[:])

            pooled = const.tile([128, KC, Q], bf16)
            for k in range(KC):
                t_ps = psm.tile([128, Q], f32, tag="m")
                for i in range(KN):
                    nc.tensor.matmul(t_ps[:],
                                     f_sb[i][:, k * 128:(k + 1) * 128],
                                     uT[:, i, :],
                                     start=(i == 0), stop=(i == KN - 1))
                eng = nc.vector if k < 2 else nc.gpsimd
                x1 = work.tile([128, Q], f32, tag="x1")
                nc.vector.tensor_scalar_add(x1[:], t_ps[:], colsum[:, k:k + 1])
                x2 = work.tile([128, Q], f32, tag="x2")
                eng.tensor_mul(x2[:], x1[:], gammaT[:, k, :])
                x3 = work.tile([128, Q], f32, tag="x3")
                eng.tensor_mul(x3[:], x2[:], a_bc[:])
                eng.tensor_add(pooled[:, k, :], x3[:], betaT[:, k, :])

            # ---------------- classification MLP ----------------
            h2 = const.tile([128, KC, Q], bf16)
            for j in range(KC):
                ps = psm.tile([128, Q], f32, tag="m")
                for k in range(KC):
                    nc.tensor.matmul(ps[:],
                                     wm1b_sb[:, k, j * 128:(j + 1) * 128],
                                     pooled[:, k, :],
                                     start=(k == 0), stop=(k == KC - 1))
                nc.scalar.activation(h2[:, j, :], ps[:], A.Gelu,
                                     bias=bm1_sb[:, j:j + 1], scale=1.0)

            lg_ps = psm.tile([Q, NCLS], f32, tag="m")
            for j in range(KC):
                nc.tensor.matmul(lg_ps[:], h2[:, j, :], wm2b_sb[:, j, :],
                                 start=(j == 0), stop=False)
            nc.tensor.matmul(lg_ps[:], ones_b[:, :Q], bm2b_sb[:],
                             start=False, stop=True)
            lg_sb = const.tile([Q, NCLS], f32)
            nc.vector.tensor_copy(lg_sb[:], lg_ps[:])
            nc.sync.dma_start(out_d[:], lg_sb[:])

    nc.compile()
    return nc


def _maybe_install_trace_shim():
    """Register the NTFF profile hook (missing antenv.axon_hooks in this
    image) so run_bass_kernel_spmd(trace=True) can return exec_time_ns."""
    try:
        import sys, types
        import antenv  # noqa: F401
        if "antenv.axon_hooks" not in sys.modules:
            mod = types.ModuleType("antenv.axon_hooks")
            mod._hook = None
            def _set(h):
                mod._hook = h
            def _get():
                return mod._hook
            mod.set_axon_ntff_profile_hook = _set
            mod.get_axon_ntff_profile_hook = _get
            sys.modules["antenv.axon_hooks"] = mod
            antenv.axon_hooks = mod
        from trn_agent_boot.trn_boot import _ntff_profile_via_ctypes
        sys.modules["antenv.axon_hooks"].set_axon_ntff_profile_hook(
            _ntff_profile_via_ctypes("/opt/axon/libaxon_pjrt.so"))
        import concourse.bass_utils as bu
        bu.upload_artifacts = lambda tmpdir: tmpdir
        return True
    except Exception:
        return False


def kernel(**inputs) -> np.ndarray:
    global LAST_EXEC_NS, LAST_TRACE
    import ml_dtypes
    from concourse.bass_utils import run_bass_kernel_spmd

    bf = ml_dtypes.bfloat16
    feature = np.asarray(inputs["feature"], dtype=np.float32)      # (B,C,H,W)
    label_ids = np.asarray(inputs["label_ids"]).astype(np.int64)   # (B,Q)
    query_table = np.asarray(inputs["query_table"], dtype=np.float32)
    W_film = np.asarray(inputs["W_film"], dtype=np.float32)
    b_film = np.asarray(inputs["b_film"], dtype=np.float32)
    W_att_h = np.asarray(inputs["W_att_h"], dtype=np.float32)
    b_att_h = np.asarray(inputs["b_att_h"], dtype=np.float32)
    W_att_f = np.asarray(inputs["W_att_f"], dtype=np.float32)
    b_att_f = np.asarray(inputs["b_att_f"], dtype=np.float32)
    W_mlp1 = np.asarray(inputs["W_mlp1"], dtype=np.float32)
    b_mlp1 = np.asarray(inputs["b_mlp1"], dtype=np.float32)
    W_mlp2 = np.asarray(inputs["W_mlp2"], dtype=np.float32)
    b_mlp2 = np.asarray(inputs["b_mlp2"], dtype=np.float32)

    if "nc" not in _CACHE:
        _CACHE["nc"] = _build_nc()
    nc = _CACHE["nc"]

    lab_range = np.arange(NLAB, dtype=np.int64)
    shared = {
        "qtb": np.ascontiguousarray(query_table.astype(bf)),
        "wfilmb": np.ascontiguousarray(W_film.astype(bf)),
        "bfilmb": np.ascontiguousarray(b_film.reshape(1, 2 * C).astype(bf)),
        "wahb": np.ascontiguousarray(W_att_h.astype(bf)),
        "bahb": np.ascontiguousarray(b_att_h.reshape(1, HID).astype(bf)),
        "wafhb": np.ascontiguousarray(
            (0.5 * W_att_f).reshape(HID, 1).astype(bf)),
        "baf": np.ascontiguousarray(b_att_f.reshape(1, 1)),
        "wm1b": np.ascontiguousarray(W_mlp1.astype(bf)),
        "bm1": np.ascontiguousarray(b_mlp1.reshape(HID2)),
        "wm2b": np.ascontiguousarray(W_mlp2.astype(bf)),
        "bm2b": np.ascontiguousarray(b_mlp2.reshape(1, NCLS).astype(bf)),
    }
    in_maps = []
    for b in range(B):
        onehot = (label_ids[b][None, :] == lab_range[:, None])
        m = dict(shared)
        m["feat"] = np.ascontiguousarray(feature[b].reshape(C, HW))
        m["ohb"] = np.ascontiguousarray(onehot.astype(bf))
        in_maps.append(m)

    trace = os.environ.get("BASS_KERNEL_TRACE", "") == "1"
    if trace:
        _maybe_install_trace_shim()
    res = run_bass_kernel_spmd(nc, in_maps, list(range(NCORES)), trace=trace,
                               tmpdir=os.environ.get("BASS_KERNEL_TMPDIR"))
    LAST_EXEC_NS = res.exec_time_ns
    if res.instructions_and_trace is not None:
        LAST_TRACE = res.instructions_and_trace[1]
    out = np.stack([res.results[i]["out"] for i in range(NCORES)], axis=0)
    return out.astype(np.float32)


# revision 11
# speedup vs baseline: 3.3575x; 3.3575x over previous
"""Trainium2 Bass kernel for nn_ClassificationHead.

Math (per batch b, query q):
  qe        = query_table[label_ids[b,q]]                      (E,)
  gamma,beta= split(qe @ W_film + b_film)                      (C,), (C,)
  film[n,c] = f[n,c]*gamma[c] + beta[c]      f = feature[b] as (HW, C)
  hidden    = gelu(film @ W_att_h + b_att_h)                   (HW, H)
  attn      = sigmoid(hidden @ W_att_f + b_att_f)              (HW, 1)
  pooled    = sum_n attn*film / (sum attn + 1e-8)              (C,)
  logits    = gelu(pooled @ W_mlp1 + b_mlp1) @ W_mlp2 + b_mlp2 (NCLS,)

Restructure used on device (film is never materialized):
  hiddenT[h,n] = gelu( sum_c (gamma[c]*W_att_h[c,h]) * fT[c,n] + bh[h] )
      with bh[h] = b_att_h[h] + sum_c beta[c]*W_att_h[c,h]
  attn = 0.5 + 0.5*u,  uT[n] = tanh( (hiddenT[:,n] . 0.5*W_att_f) + 0.5*b_att_f )
      computed directly in transposed layout: per n-chunk i a matmul with
      lhsT = hiddenT[:, i*128:(i+1)*128] writes psum column q of uT_ps[i]
  pooled[c] = gamma[c]*(colsum_f[c] + t[c]) * a  +  beta[c]
      t[c] = sum_n f[n,c]*u[n],  a = 1/(HW + sum_n u[n] + 2e-8),
      colsum_f[c] = sum_n fT[c,n]
  (the beta ratio (HW+S)/(HW+S+2e-8) rounds to exactly 1.0 in fp32, as it
   does in the fp32 reference, so it is dropped)

Sharding: data-parallel over B across the 8 cores (B == 8).
Both gelu (hidden + MLP) and tanh come from the single ACT table set
`gelu_and_others`, so there is exactly one activation-table load.
All matmul inputs are bf16 (fp32 LDWEIGHTS runs two passes and is ~6x
slower); accumulation is always fp32 in PSUM.  The per-q gamma*W scaling
is split 2 chunks on DVE + 2 on GpSimd so neither engine gates the PE.
"""

import os
import numpy as np

B = 8
Q = 64
C = 512
HW = 256
E = 512
HID = 128       # HID_ATT
HID2 = 512      # HID_MLP
NCLS = 14
NLAB = 64
NCORES = 8

_CACHE = {}
LAST_EXEC_NS = None
LAST_TRACE = None


def _build_nc():
    import concourse.bass as bass
    import concourse.tile as tile
    from concourse import bacc, mybir

    f32 = mybir.dt.float32
    bf16 = mybir.dt.bfloat16
    A = mybir.ActivationFunctionType
    AX = mybir.AxisListType

    nc = bacc.Bacc("TRN2", target_bir_lowering=False, debug=False,
                   num_devices=NCORES)

    # ---- DRAM tensors (per-core inputs) ----
    feat_d = nc.dram_tensor("feat", [C, HW], f32, kind="ExternalInput")
    ohb_d = nc.dram_tensor("ohb", [NLAB, Q], bf16, kind="ExternalInput")
    qtb_d = nc.dram_tensor("qtb", [NLAB, E], bf16, kind="ExternalInput")
    wfilmb_d = nc.dram_tensor("wfilmb", [E, 2 * C], bf16, kind="ExternalInput")
    bfilmb_d = nc.dram_tensor("bfilmb", [1, 2 * C], bf16, kind="ExternalInput")
    wahb_d = nc.dram_tensor("wahb", [C, HID], bf16, kind="ExternalInput")
    bahb_d = nc.dram_tensor("bahb", [1, HID], bf16, kind="ExternalInput")
    wafhb_d = nc.dram_tensor("wafhb", [HID, 1], bf16, kind="ExternalInput")
    baf_d = nc.dram_tensor("baf", [1, 1], f32, kind="ExternalInput")
    wm1b_d = nc.dram_tensor("wm1b", [C, HID2], bf16, kind="ExternalInput")
    bm1_d = nc.dram_tensor("bm1", [HID2], f32, kind="ExternalInput")
    wm2b_d = nc.dram_tensor("wm2b", [C, NCLS], bf16, kind="ExternalInput")
    bm2b_d = nc.dram_tensor("bm2b", [1, NCLS], bf16, kind="ExternalInput")
    out_d = nc.dram_tensor("out", [Q, NCLS], f32, kind="ExternalOutput")

    KC = C // 128   # 4 c-chunks
    KN = HW // 128  # 2 n-chunks

    with tile.TileContext(nc) as tc:
        with (
            tc.tile_pool(name="const", bufs=1) as const,
            tc.tile_pool(name="work", bufs=3) as work,
            tc.tile_pool(name="psmisc", bufs=3, space="PSUM") as psm,
            tc.tile_pool(name="pshid", bufs=2, space="PSUM") as psh,
            tc.tile_pool(name="psu", bufs=1, space="PSUM") as psu,
        ):
            # ---------------- DMAs in ----------------
            qtb_sb = const.tile([NLAB, E], bf16)
            nc.sync.dma_start(qtb_sb[:], qtb_d[:])
            ohb_sb = const.tile([NLAB, Q], bf16)
            nc.sync.dma_start(ohb_sb[:], ohb_d[:])
            bfilmb_sb = const.tile([1, 2 * C], bf16)
            nc.sync.dma_start(bfilmb_sb[:], bfilmb_d[:])
            wahb_sb = const.tile([128, KC, HID], bf16)
            nc.sync.dma_start(
                wahb_sb[:], wahb_d.ap().rearrange("(k p) h -> p k h", p=128))
            bahb_sb = const.tile([1, HID], bf16)
            nc.sync.dma_start(bahb_sb[:], bahb_d[:])
            wafhb_sb = const.tile([HID, 1], bf16)
            nc.sync.dma_start(wafhb_sb[:], wafhb_d[:])
            baf_sb = const.tile([1, 1], f32)
            nc.sync.dma_start(baf_sb[:], baf_d[:])
            wfb_sb = []
            for k in range(KC):
                t = const.tile([128, 2 * C], bf16, tag=f"wf{k}")
                nc.sync.dma_start(t[:], wfilmb_d[k * 128:(k + 1) * 128, :])
                wfb_sb.append(t)
            feat_sb = const.tile([128, KC, HW], f32)
            nc.sync.dma_start(
                feat_sb[:], feat_d.ap().rearrange("(k p) n -> p k n", p=128))
            wm1b_sb = const.tile([128, KC, HID2], bf16)
            nc.sync.dma_start(
                wm1b_sb[:], wm1b_d.ap().rearrange("(k p) m -> p k m", p=128))
            bm1_sb = const.tile([128, KC], f32)
            nc.sync.dma_start(
                bm1_sb[:], bm1_d.ap().rearrange("(k p) -> p k", p=128))
            wm2b_sb = const.tile([128, KC, NCLS], bf16)
            nc.sync.dma_start(
                wm2b_sb[:], wm2b_d.ap().rearrange("(k p) o -> p k o", p=128))
            bm2b_sb = const.tile([1, NCLS], bf16)
            nc.sync.dma_start(bm2b_sb[:], bm2b_d[:])

            # ---------------- constants ----------------
            ones_b = const.tile([1, 128], bf16)
            nc.vector.memset(ones_b[:], 1.0)
            half_row = const.tile([1, 128], f32)
            nc.vector.memset(half_row[:], 0.5)
            ones_col_b = const.tile([128, 1], bf16)
            nc.vector.memset(ones_col_b[:], 1.0)
            ones_row = const.tile([1, 128], f32)
            nc.vector.memset(ones_row[:], 1.0)

            # ---------------- setup compute ----------------
            # qe^T = query_table^T gathered by onehot: [e, q] in 4 chunks
            qeT = const.tile([128, KC, Q], bf16)
            for k in range(KC):
                ps = psm.tile([128, Q], f32, tag="m")
                nc.tensor.matmul(ps[:], qtb_sb[:, k * 128:(k + 1) * 128],
                                 ohb_sb[:], start=True, stop=True)
                nc.vector.tensor_copy(qeT[:, k, :], ps[:])

            # gb^T = W_film^T @ qe^T + b_film  -> gammaT, betaT  [c, q] (fp32)
            gammaT = const.tile([128, KC, Q], f32)
            betaT = const.tile([128, KC, Q], f32)
            for j in range(2 * KC):
                ps = psm.tile([128, Q], f32, tag="m")
                for k in range(KC):
                    nc.tensor.matmul(ps[:],
                                     wfb_sb[k][:, j * 128:(j + 1) * 128],
                                     qeT[:, k, :],
                                     start=(k == 0), stop=False)
                nc.tensor.matmul(ps[:], bfilmb_sb[:, j * 128:(j + 1) * 128],
                                 ones_b[:, :Q], start=False, stop=True)
                if j < KC:
                    nc.vector.tensor_copy(gammaT[:, j, :], ps[:])
                else:
                    nc.scalar.copy(betaT[:, j - KC, :], ps[:])
            betab = const.tile([128, KC, Q], bf16)
            nc.vector.tensor_copy(betab[:], betaT[:])
            gammab = const.tile([128, KC, Q], bf16)
            nc.vector.tensor_copy(gammab[:], gammaT[:])

            # bh[h, q] = b_att_h[h] + sum_c beta[c,q] W_att_h[c,h]
            bh_sb = const.tile([HID, Q], f32)
            ps_bh = psm.tile([HID, Q], f32, tag="m")
            for k in range(KC):
                nc.tensor.matmul(ps_bh[:], wahb_sb[:, k, :], betab[:, k, :],
                                 start=(k == 0), stop=False)
            nc.tensor.matmul(ps_bh[:], bahb_sb[:], ones_b[:, :Q],
                             start=False, stop=True)
            nc.vector.tensor_copy(bh_sb[:], ps_bh[:])

            # 0.5*b_att_f broadcast down 128 partitions
            bfa_half = const.tile([128, 1], f32)
            ps_bf = psm.tile([128, 1], f32, tag="m")
            nc.tensor.matmul(ps_bf[:], half_row[:], baf_sb[:],
                             start=True, stop=True)
            nc.vector.tensor_copy(bfa_half[:], ps_bf[:])

            # bf16 copy of f^T; exact column sums of f^T (fp32)
            ftb = const.tile([128, KC, HW], bf16)
            nc.vector.tensor_copy(ftb[:], feat_sb[:])
            colsum = const.tile([128, KC], f32)
            for k in range(KC):
                nc.vector.reduce_sum(colsum[:, k:k + 1], feat_sb[:, k, :],
                                     axis=AX.X)

            # f (HW-major, bf16) via SBUF->SBUF DMA transposes
            f_sb = []
            for i in range(KN):
                t = const.tile([128, C], bf16, tag=f"fsb{i}")
                f_sb.append(t)
            for i in range(KN):
                for k in range(KC):
                    nc.sync.dma_start_transpose(
                        f_sb[i][:, k * 128:(k + 1) * 128],
                        ftb[:, k, i * 128:(i + 1) * 128])

            # ---------------- main loop over queries ----------------
            # uT_ps[i][n, q] accumulates attention-logit columns directly in
            # transposed layout (free-dim offset q is unrestricted).
            uT_ps0 = psu.tile([128, Q], f32, tag="u0")
            uT_ps1 = psu.tile([128, Q], f32, tag="u1")
            uT_ps = [uT_ps0, uT_ps1]
            for q in range(Q):
                wq = work.tile([128, KC, HID], bf16, tag="wq")
                for k in range(KC - 1):
                    nc.vector.tensor_scalar_mul(
                        wq[:, k, :], wahb_sb[:, k, :], gammaT[:, k, q:q + 1])
                nc.scalar.activation(wq[:, KC - 1, :], wahb_sb[:, KC - 1, :],
                                     A.Copy, scale=gammaT[:, KC - 1, q:q + 1])
                hid_ps = psh.tile([HID, HW], f32, tag="h")
                for k in range(KC):
                    nc.tensor.matmul(hid_ps[:], wq[:, k, :], ftb[:, k, :],
                                     start=(k == 0), stop=(k == KC - 1))
                hidT = work.tile([HID, HW], bf16, tag="hidT")
                nc.scalar.activation(hidT[:], hid_ps[:], A.Gelu,
                                     bias=bh_sb[:, q:q + 1], scale=1.0)
                for i in range(KN):
                    nc.tensor.matmul(uT_ps[i][:, q:q + 1],
                                     hidT[:, i * 128:(i + 1) * 128],
                                     wafhb_sb[:], start=True, stop=True)

            # ---------------- pooling tail ----------------
            # u = tanh(0.5 z + 0.5 b_att_f), straight from PSUM, bf16
            uT = const.tile([128, KN, Q], bf16)
            for i in range(KN):
                nc.scalar.activation(uT[:, i, :], uT_ps[i][:], A.Tanh,
                                     bias=bfa_half[:], scale=1.0)

            # t[c, q] = sum_n f[n,c] u[n,q];  S[q] = sum_n u[n,q]
            s_ps = psm.tile([1, Q], f32, tag="m")
            for i in range(KN):
                nc.tensor.matmul(s_ps[:], ones_col_b[:], uT[:, i, :],
                                 start=(i == 0), stop=(i == KN - 1))
            den = const.tile([1, Q], f32)
            nc.vector.tensor_scalar_add(den[:], s_ps[:], float(HW) + 2e-8)
            inv = const.tile([1, 128], f32)
            nc.vector.reciprocal(inv[:, :Q], den[:])
            # broadcast a = inv down partitions
            bc_ps = psm.tile([128, Q], f32, tag="m")
            nc.tensor.matmul(bc_ps[:], ones_row[:], inv[:, :Q],
                             start=True, stop=True)
            a_bc = const.tile([128, Q], f32)
            nc.vector.tensor_copy(a_bc[:], bc_ps[:])

            pooled = const.tile([128, KC, Q], bf16)
            for k in range(KC):
                t_ps = psm.tile([128, Q], f32, tag="m")
                for i in range(KN):
                    nc.tensor.matmul(t_ps[:],
                                     f_sb[i][:, k * 128:(k + 1) * 128],
                                     uT[:, i, :],
                                     start=(i == 0), stop=(i == KN - 1))
                x1 = work.tile([128, Q], f32, tag="x1")
                nc.vector.tensor_scalar_add(x1[:], t_ps[:], colsum[:, k:k + 1])
                x2 = work.tile([128, Q], f32, tag="x2")
                nc.vector.tensor_mul(x2[:], x1[:], gammaT[:, k, :])
                x3 = work.tile([128, Q], f32, tag="x3")
                nc.vector.tensor_mul(x3[:], x2[:], a_bc[:])
                nc.vector.tensor_add(pooled[:, k, :], x3[:], betaT[:, k, :])

            # ---------------- classification MLP ----------------
            h2 = const.tile([128, KC, Q], bf16)
            for j in range(KC):
                ps = psm.tile([128, Q], f32, tag="m")
                for k in range(KC):
                    nc.tensor.matmul(ps[:],
                                     wm1b_sb[:, k, j * 128:(j + 1) * 128],
                                     pooled[:, k, :],
                                     start=(k == 0), stop=(k == KC - 1))
                nc.scalar.activation(h2[:, j, :], ps[:], A.Gelu,
                                     bias=bm1_sb[:, j:j + 1], scale=1.0)

            lg_ps = psm.tile([Q, NCLS], f32, tag="m")
            for j in range(KC):
                nc.tensor.matmul(lg_ps[:], h2[:, j, :], wm2b_sb[:, j, :],
                                 start=(j == 0), stop=False)
            nc.tensor.matmul(lg_ps[:], ones_b[:, :Q], bm2b_sb[:],
                             start=False, stop=True)
            lg_sb = const.tile([Q, NCLS], f32)
            nc.vector.tensor_copy(lg_sb[:], lg_ps[:])
            nc.sync.dma_start(out_d[:], lg_sb[:])

    nc.compile()
    return nc


def _maybe_install_trace_shim():
    """Register the NTFF profile hook (missing antenv.axon_hooks in this
    image) so run_bass_kernel_spmd(trace=True) can return exec_time_ns."""
    try:
        import sys, types
        import antenv  # noqa: F401
        if "antenv.axon_hooks" not in sys.modules:
            mod = types.ModuleType("antenv.axon_hooks")
            mod._hook = None
            def _set(h):
                mod._hook = h
            def _get():
                return mod._hook
            mod.set_axon_ntff_profile_hook = _set
            mod.get_axon_ntff_profile_hook = _get
            sys.modules["antenv.axon_hooks"] = mod
            antenv.axon_hooks = mod
        from trn_agent_boot.trn_boot import _ntff_profile_via_ctypes
        sys.modules["antenv.axon_hooks"].set_axon_ntff_profile_hook(
            _ntff_profile_via_ctypes("/opt/axon/libaxon_pjrt.so"))
        import concourse.bass_utils as bu
        bu.upload_artifacts = lambda tmpdir: tmpdir
        return True
    except Exception:
        return False


def kernel(**inputs) -> np.ndarray:
    global LAST_EXEC_NS, LAST_TRACE
    import ml_dtypes
    from concourse.bass_utils import run_bass_kernel_spmd

    bf = ml_dtypes.bfloat16
    feature = np.asarray(inputs["feature"], dtype=np.float32)      # (B,C,H,W)
    label_ids = np.asarray(inputs["label_ids"]).astype(np.int64)   # (B,Q)
    query_table = np.asarray(inputs["query_table"], dtype=np.float32)
    W_film = np.asarray(inputs["W_film"], dtype=np.float32)
    b_film = np.asarray(inputs["b_film"], dtype=np.float32)
    W_att_h = np.asarray(inputs["W_att_h"], dtype=np.float32)
    b_att_h = np.asarray(inputs["b_att_h"], dtype=np.float32)
    W_att_f = np.asarray(inputs["W_att_f"], dtype=np.float32)
    b_att_f = np.asarray(inputs["b_att_f"], dtype=np.float32)
    W_mlp1 = np.asarray(inputs["W_mlp1"], dtype=np.float32)
    b_mlp1 = np.asarray(inputs["b_mlp1"], dtype=np.float32)
    W_mlp2 = np.asarray(inputs["W_mlp2"], dtype=np.float32)
    b_mlp2 = np.asarray(inputs["b_mlp2"], dtype=np.float32)

    if "nc" not in _CACHE:
        _CACHE["nc"] = _build_nc()
    nc = _CACHE["nc"]

    lab_range = np.arange(NLAB, dtype=np.int64)
    shared = {
        "qtb": np.ascontiguousarray(query_table.astype(bf)),
        "wfilmb": np.ascontiguousarray(W_film.astype(bf)),
        "bfilmb": np.ascontiguousarray(b_film.reshape(1, 2 * C).astype(bf)),
        "wahb": np.ascontiguousarray(W_att_h.astype(bf)),
        "bahb": np.ascontiguousarray(b_att_h.reshape(1, HID).astype(bf)),
        "wafhb": np.ascontiguousarray(
            (0.5 * W_att_f).reshape(HID, 1).astype(bf)),
        "baf": np.ascontiguousarray(b_att_f.reshape(1, 1)),
        "wm1b": np.ascontiguousarray(W_mlp1.astype(bf)),
        "bm1": np.ascontiguousarray(b_mlp1.reshape(HID2)),
        "wm2b": np.ascontiguousarray(W_mlp2.astype(bf)),
        "bm2b": np.ascontiguousarray(b_mlp2.reshape(1, NCLS).astype(bf)),
    }
    in_maps = []
    for b in range(B):
        onehot = (label_ids[b][None, :] == lab_range[:, None])
        m = dict(shared)
        m["feat"] = np.ascontiguousarray(feature[b].reshape(C, HW))
        m["ohb"] = np.ascontiguousarray(onehot.astype(bf))
        in_maps.append(m)

    trace = os.environ.get("BASS_KERNEL_TRACE", "") == "1"
    if trace:
        _maybe_install_trace_shim()
    res = run_bass_kernel_spmd(nc, in_maps, list(range(NCORES)), trace=trace,
                               tmpdir=os.environ.get("BASS_KERNEL_TMPDIR"))
    LAST_EXEC_NS = res.exec_time_ns
    if res.instructions_and_trace is not None:
        LAST_TRACE = res.instructions_and_trace[1]
    out = np.stack([res.results[i]["out"] for i in range(NCORES)], axis=0)
    return out.astype(np.float32)
